# revision 1
# baseline (speedup 1.0000x reference)
"""Trainium2 Bass kernel for 2D single-level DWT (coif1, symmetric padding).

Input  x: (4, 64, 512, 512) fp32
Output  : (4, 256, 258, 258) fp32  -- per input channel: [cA, cH, cV, cD]

Math: with R_f the banded 258x512 operator of the 1D DWT along an axis
(6-tap filter, stride 2, symmetric boundary folds), the four outputs are
    cA = R_lo X R_lo^T,  cH = R_hi X R_lo^T,
    cV = R_lo X R_hi^T,  cD = R_hi X R_hi^T.

On-device (per image, per core; 32 images per core, pure data-parallel):
  pass 1 (contract over rows r on the PE):   Yt_f[c, kh] = sum_r X[r, c] R_f[kh, r]
     matmul with lhsT = X column-slice (stationary), rhs = R_f^T chunk.
  pass 2 (contract over cols c on the PE):   O_s[kw, kh] = sum_c R_g[kw, c] Yt_f[c, kh]
     matmul with lhsT = R_g^T kw-slice (stationary), rhs = Yt_f chunk.
  Outputs land transposed ([kw, kh]); the host swaps the last two axes.

Matmuls run as float32r (1 cycle/row for N>=256, numerically fp32-width).
"""

import os
import sys

for _p in ("/opt/trn_rl_repo", "/opt/pypackages"):
    if _p not in sys.path:
        sys.path.append(_p)

os.environ.setdefault("JAX_COMPILATION_CACHE_DIR", "/tmp/jax_comp_cache")
os.environ.setdefault("JAX_PERSISTENT_CACHE_MIN_COMPILE_TIME_SECS", "10")

import numpy as np

import concourse.bass as bass
import concourse.bacc as bacc
import concourse.mybir as mybir
from concourse.bass_utils import run_bass_kernel_spmd
from concourse.tile import TileContext

N_CORES = 8
H = W = 512
OUT = 258  # (512 + 6 - 1) // 2
IMGS = 32  # images per core (4*64/8)
F32 = mybir.dt.float32
F32R = mybir.dt.float32r

# pywt coif1 decomposition filters (already flipped: correlation form)
DEC_LO = np.array([-0.01565572813546454, -0.0727326195128539, 0.38486484686420286,
                   0.8525720202122554, 0.3378976624578092, -0.0727326195128539])
DEC_HI = np.array([0.0727326195128539, 0.3378976624578092, -0.8525720202122554,
                   0.38486484686420286, 0.0727326195128539, -0.01565572813546454])
FLEN = 6
PAD = 4
LO_F = DEC_LO[::-1]
HI_F = DEC_HI[::-1]


def _build_R(filt: np.ndarray, n: int = W) -> np.ndarray:
    """Banded [258, 512] operator: out[k] = sum_j filt[j] * x[sym(2k + j - PAD)]."""
    out_len = (n + FLEN - 1) // 2

    def sym(i: int) -> int:
        while i < 0 or i >= n:
            if i < 0:
                i = -i - 1
            if i >= n:
                i = 2 * n - 1 - i
        return i

    R = np.zeros((out_len, n), dtype=np.float64)
    for k in range(out_len):
        for j in range(FLEN):
            R[k, sym(2 * k + j - PAD)] += filt[j]
    return R


def _build_weights() -> np.ndarray:
    """w[p, (f*4+q)*258 + k] = R_f[k, 128q + p], as [128, 8*258] fp32."""
    Rs = [_build_R(LO_F), _build_R(HI_F)]
    tiles = []
    for f in range(2):
        for q in range(4):
            tiles.append(Rs[f][:, 128 * q:128 * (q + 1)].T)
    stacked = np.stack(tiles)  # [8, 128, 258]
    w = np.ascontiguousarray(stacked.transpose(1, 0, 2).reshape(128, 8 * OUT)).astype(np.float32)
    return _round_tf32(w)


def _round_tf32(a: np.ndarray) -> np.ndarray:
    """Round-to-nearest-even to tf32 (10-bit mantissa), keeping fp32 storage."""
    bits = a.astype(np.float32).view(np.uint32)
    bits = (bits + 0xFFF + ((bits >> 13) & 1)) & np.uint32(0xFFFFE000)
    return bits.view(np.float32)


_WEIGHTS = _build_weights()
_MODULE = None
PS1_BUFS = 3
PS2_BUFS = 5
W_RING_SCALAR = False
XPOOL_BUFS = 2
YPOOL_BUFS = 2
SPOOL_BUFS = 2
OUT_SPLIT = 4
IN_SPLIT = 1


def _build_module() -> bass.Bass:
    nc = bacc.Bacc("TRN2", target_bir_lowering=False, debug=False)
    x_in = nc.declare_dram_parameter("x", [IMGS, H, W], F32R, isOutput=False)
    w_in = nc.declare_dram_parameter("w", [128, 8 * OUT], F32R, isOutput=False)
    # device layout: y[i, s, kw, kh] = O_s[kw, kh] (host swaps kh/kw)
    y_out = nc.declare_dram_parameter("y", [IMGS, 4, OUT, OUT], F32, isOutput=True)

    with TileContext(nc) as tc:
        with (
            tc.tile_pool(name="wpool", bufs=1) as wpool,
            tc.tile_pool(name="xpool", bufs=XPOOL_BUFS) as xpool,
            tc.tile_pool(name="ypool", bufs=YPOOL_BUFS) as ypool,
            tc.tile_pool(name="spool", bufs=SPOOL_BUFS) as spool,
            tc.tile_pool(name="psum", bufs=4, space="PSUM") as pspool,
        ):
            Wt = wpool.tile([128, 8 * OUT], F32R)
            # scalar-ring HWDGE so the weight load overlaps the first X load
            (nc.scalar if W_RING_SCALAR else nc.sync).dma_start(out=Wt[:], in_=w_in[:])
            Wr = Wt[:]

            # Tiny PE op consuming the weight DMA so later matmuls depend on
            # it via PE program order (Matmult carries at most one sync wait).
            warm = pspool.tile([1, OUT], F32, tag="ps2", bufs=PS2_BUFS)
            nc.tensor.matmul(warm[:, :], lhsT=Wr[:, 0:1], rhs=Wr[:, 0:OUT],
                             start=True, stop=True)

            def load_x(i):
                # X[p, q*512 + c] = x[i, 128q + p, c]
                X = xpool.tile([128, 4 * W], F32R, tag="X", name=f"X_{i}")
                xi = x_in[i].rearrange("(q p) c -> p q c", p=128)
                Xv = X.rearrange("p (q c) -> p q c", q=4)
                qper = 4 // IN_SPLIT
                for j in range(IN_SPLIT):
                    nc.sync.dma_start(
                        out=Xv[:, j * qper:(j + 1) * qper],
                        in_=xi[:, j * qper:(j + 1) * qper],
                    )
                return X

            ev = 0
            Xnext = load_x(0)
            for i in range(IMGS):
                Xr = Xnext[:]

                # pass 1: Yt[p, (f*4+cc)*258 + kh] = Yt_f[c = 128cc + p, kh]
                Yt = ypool.tile([128, 8 * OUT], F32R, tag="Yt")
                for f in range(2):
                    for cc in range(4):
                        ps = pspool.tile([128, OUT], F32, tag="ps1", bufs=PS1_BUFS)
                        for q in range(4):
                            nc.tensor.matmul(
                                ps[:, :],
                                lhsT=Xr[:, q * W + cc * 128: q * W + (cc + 1) * 128],
                                rhs=Wr[:, (f * 4 + q) * OUT: (f * 4 + q + 1) * OUT],
                                start=(q == 0),
                                stop=(q == 3),
                            )
                        dst = Yt[:, (f * 4 + cc) * OUT: (f * 4 + cc + 1) * OUT]
                        if ev % 2 == 0:
                            nc.scalar.copy(out=dst, in_=ps[:, :])
                        else:
                            nc.vector.tensor_copy(out=dst, in_=ps[:, :])
                        ev += 1
                Ytr = Yt[:]

                # prefetch the next image's input ahead of this image's stores
                # in the sync-ring FIFO
                if i + 1 < IMGS:
                    Xnext = load_x(i + 1)

                # pass 2: STG[p, (s*3+m)*258 + kh] = O_s[kw = 86m + p, kh]
                STG = spool.tile([86, 12 * OUT], F32, tag="STG")
                for g in range(2):
                    for f in range(2):
                        s = f + 2 * g
                        for m in range(3):
                            ps2 = pspool.tile([86, OUT], F32, tag="ps2", bufs=PS2_BUFS)
                            for j, q in enumerate((m, m + 1)):
                                nc.tensor.matmul(
                                    ps2[:, :],
                                    lhsT=Wr[:, (g * 4 + q) * OUT + m * 86:
                                            (g * 4 + q) * OUT + (m + 1) * 86],
                                    rhs=Ytr[:, (f * 4 + q) * OUT: (f * 4 + q + 1) * OUT],
                                    start=(j == 0),
                                    stop=(j == 1),
                                )
                            dst = STG[:, (s * 3 + m) * OUT: (s * 3 + m + 1) * OUT]
                            if ev % 2 == 0:
                                nc.scalar.copy(out=dst, in_=ps2[:, :])
                            else:
                                nc.vector.tensor_copy(out=dst, in_=ps2[:, :])
                            ev += 1

                for s in range(4):
                    nc.sync.dma_start(
                        out=y_out[i, s].rearrange("(m p) k -> p m k", p=86),
                        in_=STG[:, s * 3 * OUT:(s + 1) * 3 * OUT].rearrange(
                            "p (m k) -> p m k", m=3),
                    )
    nc.finalize()
    return nc


def _get_module() -> bass.Bass:
    global _MODULE
    if _MODULE is None:
        _MODULE = _build_module()
    return _MODULE


def kernel(**inputs) -> np.ndarray:
    x = np.asarray(inputs["x"], dtype=np.float32)
    B, C, Hx, Wx = x.shape
    assert (Hx, Wx) == (H, W) and B * C == N_CORES * IMGS
    imgs = x.reshape(B * C, H, W)

    nc = _get_module()
    in_maps = [
        {"x": _round_tf32(imgs[k * IMGS:(k + 1) * IMGS]), "w": _WEIGHTS}
        for k in range(N_CORES)
    ]
    res = run_bass_kernel_spmd(nc, in_maps, list(range(N_CORES))).results

    full = np.concatenate([res[k]["y"] for k in range(N_CORES)], axis=0)
    # device layout is [img, s, kw, kh] -> swap to [img, s, kh, kw]
    full = full.transpose(0, 1, 3, 2)
    return np.ascontiguousarray(full.reshape(B, 4 * C, OUT, OUT)).astype(np.float32)

